# revision 6
# baseline (speedup 1.0000x reference)
"""Trainium2 kernel for CondensedLinearFineGrainedSparseOp:
    out[b,s,o] = sum_k x[b,s,k] * weight[o,k] + bias[o]
with x [8, 2048, 4096] f32, weight [4096, 4096] f32 (90% zeros, stored
dense), bias [4096] f32 -> out [8, 2048, 4096] f32.

Strategy: data-parallel shard over tokens (B*S = 16384 -> 2048 per core)
across 8 NeuronCores; weight/bias replicated. The unstructured 10%
sparsity is not exploitable on the 128x128 PE array (any >=8x8 block of
the mask is nonempty with overwhelming probability), so each core runs a
dense [2048 x 4096 x 4096] GEMM in bf16 with fp32 PSUM accumulation.
PE roofline: 16t*32k*4096o cols @ 0.4167ns = 874us/core.

Schedule: k-outer with t-tiles interleaved. Each o-phase processes t-tile
groups (4 t's for 512-wide phases, 2 t's for 1024-wide); within a group
the k-loop issues one matmul per (t, 512-col bank) so a W k-tile is
consumed every 4 matmuls (~1.7us) while the HBM stream delivers one every
~0.9us -> the PE never starves on the cold first group, and per-t-tile
first-matmul stalls expose once per group instead of once per t.

x^T tiles are staged as half-tiles (k 0:16 / 16:32, 512KB DMAs) on the
SWDGE queue; the very first group's halves arrive as quarter-chunks
interleaved across its 4 t-tiles so matmul (k=0,t=0) needs only 256KB of
x + one 128KB W tile. W streams on the two HWDGE queues (even k on sync,
odd on scalar) in phase order; wpool's FIFO WAR deps head-of-line block
the rings so later phases can't steal cold-start bandwidth. Outputs and
bias use the vector engine's queue, keeping the W rings clean.
"""

import numpy as np
import ml_dtypes

import concourse.mybir as mybir
import concourse.tile as tile
from concourse import bacc
from concourse.bass import ts
from concourse.bass_utils import run_bass_kernel_spmd

P = 128
NCORES = 8
B, S, DIN, DOUT = 8, 2048, 4096, 4096
T = B * S // NCORES          # tokens per core
KT = DIN // P                # 32 contraction tiles
KH = KT // 2                 # half-tile k extent
NT = T // P                  # 16 token tiles per core
BANK = 512                   # PSUM bank width (f32)
OBLK = 1024                  # W pool slot width

# (o0, olen, interleave): t-tiles processed per k-sweep. 512-wide phases
# interleave 4 t's (1 PSUM bank each), 1024-wide phases 2 t's (2 banks).
PHASE_PLAN = [
    (0, 512, 4),
    (512, 512, 4),
    (1024, 1024, 2),
    (2048, 1024, 2),
    (3072, 1024, 2),
]

BF16 = mybir.dt.bfloat16
F32 = mybir.dt.float32

_NC = None
LAST_RESULT = None


def _build_nc():
    nc = bacc.Bacc("TRN2", target_bir_lowering=False, debug=False)
    # x pre-tiled on host: xt[t, p, ks, i] = x[t*128+i, ks*128+p] -> any
    # (ks-range) slice of a t-tile is a clean strided DMA with 2KB+ lines
    xt = nc.dram_tensor("xt", [NT, P, KT, P], BF16, kind="ExternalInput")
    wt = nc.dram_tensor("wt", [DIN, DOUT], BF16, kind="ExternalInput")
    bias = nc.dram_tensor("bias_rep", [P, DOUT], F32, kind="ExternalInput")
    out = nc.dram_tensor("out", [T, DOUT], F32, kind="ExternalOutput")

    with tile.TileContext(nc) as tc:
        with (
            tc.tile_pool(name="wpool", bufs=56) as wpool,
            tc.tile_pool(name="xpool", bufs=12) as xpool,
            tc.tile_pool(name="bpool", bufs=1) as bpool,
            tc.tile_pool(name="opool", bufs=3) as opool,
            tc.tile_pool(name="psum", bufs=8, space="PSUM") as psum_pool,
        ):
            # Tiny warmup DMA on each queue: absorbs cold DGE/queue init
            # and first-completion latency on throwaway transfers.
            for i, eng in enumerate((nc.sync, nc.scalar, nc.gpsimd)):
                wu = bpool.tile([P, 8], F32, tag=f"wu{i}", name=f"wu{i}")
                eng.dma_start(wu[:], bias.ap()[:, ts(i, 8)])

            bias_sb = bpool.tile([P, DOUT], F32)

            # --- x half-tile staging -------------------------------------
            # halves[t] = (A, B): A covers k-tiles 0:16, B covers 16:32.
            halves = {}

            def alloc_half(t, h, chunks=1):
                xtile = xpool.tile([P, KH, P], BF16, tag="x",
                                   name=f"x{t}{'ab'[h]}")
                halves.setdefault(t, [None, None])[h] = xtile
                return xtile

            # Cold first group (t 0..3): A-halves as quarter-chunks
            # interleaved across t so chunk-rows complete together.
            g0 = list(range(PHASE_PLAN[0][2]))
            for t in g0:
                alloc_half(t, 0)
            for c in range(2):           # A = k-chunks [0:8), [8:16)
                for t in g0:
                    nc.gpsimd.dma_start(
                        halves[t][0][:, ts(c, 8), :],
                        xt.ap()[t, :, ts(c, 8), :],
                    )
            for t in g0:
                alloc_half(t, 1)
            for c in range(2):           # B = k-chunks [16:24), [24:32)
                for t in g0:
                    nc.gpsimd.dma_start(
                        halves[t][1][:, ts(c, 8), :],
                        xt.ap()[t, :, ts(2 + c, 8), :],
                    )

            def prefetch(t, h):
                xtile = alloc_half(t, h)
                nc.gpsimd.dma_start(
                    xtile[:], xt.ap()[t, :, ts(h, KH), :]
                )

            # Linear group schedule across all phases.
            groups = []
            for o0, olen, ilv in PHASE_PLAN:
                for tg in range(0, NT, ilv):
                    groups.append((o0, olen, list(range(tg, tg + ilv))))

            w_tiles = None
            cur_phase = None
            for gi, (o0, olen, tlist) in enumerate(groups):
                nb = olen // BANK
                if (o0, olen) != cur_phase:
                    cur_phase = (o0, olen)
                    # W stream for this phase: even k on sync, odd on
                    # scalar; ring FIFO + pool WAR throttle later phases.
                    w_tiles = []
                    for k in range(KT):
                        wtile = wpool.tile(
                            [P, olen], BF16, tag="w", name="w",
                            padded_shape=[P, OBLK],
                        )
                        eng = nc.sync if k % 2 == 0 else nc.scalar
                        eng.dma_start(
                            wtile[:], wt.ap()[ts(k, P), o0:o0 + olen]
                        )
                        w_tiles.append(wtile)
                    # bias chunk for this phase rides the x queue (small,
                    # and lands after the cold x chunks for phase 0)
                    nc.gpsimd.dma_start(
                        bias_sb[:, o0:o0 + olen], bias.ap()[:, o0:o0 + olen]
                    )

                # prefetch x halves for the NEXT group (B of current group
                # was already requested when it was "next"); issue order:
                # nothing needed for current group here.
                if gi + 1 < len(groups):
                    for t in groups[gi + 1][2]:
                        if t not in halves or halves[t][0] is None:
                            prefetch(t, 0)
                            prefetch(t, 1)

                accs = {
                    t: [
                        psum_pool.tile([P, BANK], F32, tag="acc", name="acc")
                        for _ in range(nb)
                    ]
                    for t in tlist
                }
                for k in range(KT):
                    h, kk = divmod(k, KH)
                    for t in tlist:
                        stat = halves[t][h][:, kk, :]
                        for b in range(nb):
                            nc.tensor.matmul(
                                accs[t][b][:],
                                stat,                          # stationary
                                w_tiles[k][:, ts(b, BANK)],    # moving
                                start=(k == 0),
                                stop=(k == KT - 1),
                            )
                for t in tlist:
                    osb = opool.tile(
                        [P, olen], F32, tag="o", name="o",
                        padded_shape=[P, OBLK],
                    )
                    for b in range(nb):
                        nc.vector.tensor_add(
                            osb[:, ts(b, BANK)],
                            accs[t][b][:],
                            bias_sb[:, o0 + b * BANK:o0 + (b + 1) * BANK],
                        )
                    # alternate the two HWDGE rings; outs sit ahead of the
                    # next phase's W triggers in ring order, so wpool's WAR
                    # head-of-line blocking never delays an output write
                    oeng = nc.sync if t % 2 == 0 else nc.scalar
                    oeng.dma_start(
                        out.ap()[ts(t, P), o0:o0 + olen], osb[:]
                    )
                # current group's halves fully consumed -> drop refs
                for t in tlist:
                    halves[t] = [None, None]
                    del halves[t]

    nc.compile()
    return nc


def kernel(x, weight, bias):
    global _NC, LAST_RESULT
    if _NC is None:
        _NC = _build_nc()

    X = np.ascontiguousarray(x.reshape(B * S, DIN))
    wt = weight.T.astype(ml_dtypes.bfloat16)          # [k, o] bf16
    bias_rep = np.ascontiguousarray(
        np.broadcast_to(bias.astype(np.float32), (P, DOUT))
    )
    in_maps = []
    for c in range(NCORES):
        xc = X[c * T : (c + 1) * T].astype(ml_dtypes.bfloat16)
        # [t-tile, p(=k%128), ks, i(=token%128)]
        xt_c = np.ascontiguousarray(
            xc.reshape(NT, P, KT, P).transpose(0, 3, 2, 1)
        )
        in_maps.append({"xt": xt_c, "wt": wt, "bias_rep": bias_rep})

    last_err = None
    for _attempt in range(2):
        try:
            res = run_bass_kernel_spmd(_NC, in_maps, list(range(NCORES)))
            break
        except Exception as e:  # transient NRT device errors: retry once
            last_err = e
    else:
        raise last_err
    LAST_RESULT = res

    out = np.concatenate([res.results[c]["out"] for c in range(NCORES)], axis=0)
    return out.reshape(B, S, DOUT).astype(np.float32, copy=False)
